# revision 21
# baseline (speedup 1.0000x reference)
"""Trainium2 Bass kernel for nn_AdaptiveSubgraphLayer (hyperbolic GNN + PNA).

v3 node-parallel layout (8 NeuronCores, SPMD):
  - Host (int-only): per core, nodes sorted by degree desc, batches of 128
    nodes (node -> partition). Batch j owns Lb[j] tile columns; tile l holds
    edge l of each of the 128 nodes (padding duplicates edge 0, corrected in
    the sums; deg-0/fake nodes hit the zero rel-row with valid=0 -> msg 0).
    Batch structure identical across cores.
  - Device per tile: indirect-gather hidden rows (slot-major), one-hot
    matmul expands ExpR[rel] (+|y|^2 col) into PSUM-bf16. All hyperbolic
    maps collapse to polynomials in (|u|^2, u.B, |B|^2) -- tanh and atanh
    are replaced by odd-series in the squared norms (valid: |m| < 0.6),
    so no activation tables in the hot loop. msg = fA*u + fB*B.
  - Segment sums via identity-matmul PSUM accumulation over the batch's
    tiles; max/min via pairwise-tree tensor_tensor over the batch buffer.
  - Batch epilogue: PNA mean/std/max/min node-major, 5 PE transposes,
    W_agg matmuls (384-wide rhs), scaler fold; h_tilde stays in SBUF.
  - Tiny allreduce shares the 8 user rows; sigmoid gate at the end.
"""
import sys
import os

sys.path.insert(0, "/opt/trn_rl_repo")
sys.path.insert(0, os.path.dirname(os.path.abspath(__file__)))

import numpy as np

N_NODES, N_PREV, N_EDGES, DIM, BATCH = 100000, 80000, 1000000, 128, 8
N_REL = 43
NCORES = 8
P = 128
SG = 28
MIN_NORM = 1e-15


# --- inlined walrus single-wait workaround (kernel.py must be self-contained) ---
_TILE_PATCH_SRC = '"""Workaround: the walrus build in this container supports only ONE sem-wait\nper ISA instruction; Tile\'s scheduler attaches several. After TileContext\nlowering (including the tail drain/barrier), sweep every basic block and move\nexcess waits onto same-engine nop instructions inserted immediately before\nthe over-subscribed instruction."""\nimport concourse.mybir as mybir\nfrom concourse.tile import TileContext\n\nMAX_WAITS = 1\nCOMPUTE_MAX_WAITS = 1\n_CTRL = ("InstNoOp", "InstDrain", "InstEventSemOp")\n\n\ndef _limit(inst):\n    return MAX_WAITS if type(inst).__name__ in _CTRL else COMPUTE_MAX_WAITS\n\n_orig_drain = TileContext._drain_and_barrier\n\n\ndef _split_all_waits(nc):\n    for bb in nc.main_func.blocks:\n        insts = list(bb.instructions)\n        need = []\n        for inst in insts:\n            si = inst.sync_info\n            if si is not None and len(si.on_wait) > _limit(inst):\n                need.append(inst)\n        if not need:\n            continue\n        patch = {}\n        created = []\n        for inst in need:\n            si = inst.sync_info\n            lim = _limit(inst)\n            waits = list(si.on_wait)\n            si.on_wait = waits[-lim:]\n            rest = waits[:-lim]\n            nops = []\n            eng = nc.engines[inst.engine]\n            for j in range(0, len(rest), MAX_WAITS):\n                nop = eng.nop(nofuse=True)\n                nsi = nop.ins.sync_info\n                if nsi is None:\n                    nop.ins.sync_info = mybir.SyncInfo(\n                        on_wait=rest[j:j + MAX_WAITS], on_update=[])\n                else:\n                    nsi.on_wait = rest[j:j + MAX_WAITS]\n                nops.append(nop.ins)\n                created.append(nop.ins)\n            patch[id(inst)] = nops\n        created_ids = {id(x) for x in created}\n        for bb2 in nc.main_func.blocks:\n            if any(id(x) in created_ids for x in bb2.instructions):\n                bb2.instructions[:] = [\n                    x for x in bb2.instructions if id(x) not in created_ids]\n        out = []\n        for inst in insts:\n            if id(inst) in patch:\n                out.extend(patch[id(inst)])\n            out.append(inst)\n        bb.instructions[:] = out\n\n\ndef _drain_and_barrier(self, tick_clock, wait_clock):\n    _orig_drain(self, tick_clock, wait_clock)\n    _split_all_waits(self.nc)\n\n\ndef install():\n    TileContext._drain_and_barrier = _drain_and_barrier\n'


def _install_tile_patch():
    import types, sys as _sys
    if "tile_patch" in _sys.modules:
        return _sys.modules["tile_patch"]
    m = types.ModuleType("tile_patch")
    exec(_TILE_PATCH_SRC, m.__dict__)
    _sys.modules["tile_patch"] = m
    return m


def _ensure_ntff_hook():
    try:
        import antenv.axon_hooks  # noqa: F401
        return
    except ImportError:
        pass
    import types, sys as _sys
    m = types.ModuleType("antenv.axon_hooks")
    m._hook = None
    def set_axon_ntff_profile_hook(h):
        m._hook = h
    def get_axon_ntff_profile_hook():
        return m._hook
    m.set_axon_ntff_profile_hook = set_axon_ntff_profile_hook
    m.get_axon_ntff_profile_hook = get_axon_ntff_profile_hook
    _sys.modules["antenv.axon_hooks"] = m
    try:
        import antenv
        antenv.axon_hooks = m
    except ImportError:
        pass
    try:
        from trn_agent_boot.trn_boot import _ntff_profile_via_ctypes
        hook = _ntff_profile_via_ctypes("/opt/axon/libaxon_pjrt.so")
        if hook is not None:
            m._hook = hook
    except Exception:
        pass


# ----------------------------------------------------------------------------
# host preprocessing (integers only)
# ----------------------------------------------------------------------------

def preprocess(edges, nodes, q_sub, old_nodes_new_idx):
    sub = np.asarray(edges[:, 4], dtype=np.int64)
    rel = np.asarray(edges[:, 2], dtype=np.int64)
    obj = np.asarray(edges[:, 5], dtype=np.int64)
    deg = np.bincount(obj, minlength=N_NODES)

    cum = np.cumsum(deg)
    bounds = [0] + [int(np.searchsorted(cum, N_EDGES * c / NCORES))
                    for c in range(1, NCORES)] + [N_NODES]

    order = np.argsort(obj, kind="stable")
    sub_s, rel_s = sub[order], rel[order]
    estart = np.zeros(N_NODES + 1, dtype=np.int64)
    estart[1:] = cum

    core_nodes = []
    for c in range(NCORES):
        nid = np.arange(bounds[c], bounds[c + 1])
        o = np.argsort(-deg[nid], kind="stable")
        core_nodes.append(nid[o])

    NB = max((len(n) + P - 1) // P for n in core_nodes)
    Lb = np.ones(NB, dtype=np.int64)
    for c in range(NCORES):
        dd = deg[core_nodes[c]]
        for j in range((len(dd) + P - 1) // P):
            Lb[j] = max(Lb[j], dd[j * P:(j + 1) * P].max(initial=1))
    bstart = np.zeros(NB + 1, dtype=np.int64)
    bstart[1:] = np.cumsum(Lb)
    NT = int(bstart[-1])

    inv = np.full(N_NODES, 2 ** 30, dtype=np.int64)
    inv[np.asarray(old_nodes_new_idx, dtype=np.int64)] = np.arange(N_PREV)

    nb_arr = np.asarray(nodes[:, 0], dtype=np.int64)
    ne_arr = np.asarray(nodes[:, 1], dtype=np.int64)
    user_idx = np.zeros(BATCH, dtype=np.int64)
    for b in range(BATCH):
        m = np.where((nb_arr == b) & (ne_arr == np.asarray(q_sub)[b]))[0]
        user_idx[b] = m[0]

    per_core = []
    for c in range(NCORES):
        cn = core_nodes[c]
        ncn = len(cn)
        ssub = np.zeros((P, NT), dtype=np.int32)
        srel = np.full((P, NT), N_REL, dtype=np.int64)
        valid = np.zeros((P, NT), dtype=np.float32)
        node_id = np.full(NB * P, -1, dtype=np.int64)
        degc = np.zeros((P, NB), dtype=np.float32)
        corr = np.zeros((P, NB), dtype=np.float32)
        hpidx = np.zeros((P, NB), dtype=np.int32)
        hpmask = np.zeros((P, NB), dtype=np.float32)
        for j in range(NB):
            l0, l1 = int(bstart[j]), int(bstart[j + 1])
            L = l1 - l0
            for p in range(P):
                k = j * P + p
                if k >= ncn:
                    continue
                n = int(cn[k])
                node_id[k] = n
                d = int(deg[n])
                degc[p, j] = d
                if inv[n] < N_PREV:
                    hpidx[p, j] = inv[n]
                    hpmask[p, j] = 1.0
                if d == 0:
                    continue
                corr[p, j] = d - L
                e0 = int(estart[n])
                idx = np.arange(L)
                idx[d:] = 0
                ssub[p, l0:l1] = sub_s[e0 + idx]
                srel[p, l0:l1] = rel_s[e0 + idx]
                valid[p, l0:l1] = 1.0
        per_core.append(dict(ssub=ssub, srel=srel, valid=valid,
                             node_id=node_id, deg=degc, corr=corr,
                             hpidx=hpidx, hpmask=hpmask))

    struct = dict(NB=NB, NT=NT, S=NT * P, Lb=Lb, bstart=bstart, bounds=bounds)
    return struct, per_core, inv, user_idx, nb_arr


# ----------------------------------------------------------------------------
# bass graph
# ----------------------------------------------------------------------------

def build_graph(struct):
    import concourse.bass as bass
    import concourse.mybir as mybir
    from concourse.tile import TileContext
    from concourse.masks import make_identity
    _install_tile_patch().install()

    F32, BF16, I32 = mybir.dt.float32, mybir.dt.bfloat16, mybir.dt.int32
    AF = mybir.ActivationFunctionType
    ALU = mybir.AluOpType

    NB, NT, Lb, bstart = struct["NB"], struct["NT"], struct["Lb"], struct["bstart"]
    S = struct["S"]
    LMAX = int(Lb.max())
    # tile -> (batch, pos, is_first, is_last)
    t2b = np.zeros(NT, dtype=np.int64)
    t2pos = np.zeros(NT, dtype=np.int64)
    for j in range(NB):
        t2b[bstart[j]:bstart[j + 1]] = j
        t2pos[bstart[j]:bstart[j + 1]] = np.arange(bstart[j + 1] - bstart[j])

    nc = bass.Bass()
    t_hid = nc.declare_dram_parameter("hidden", [N_PREV, DIM], F32, isOutput=False)
    t_ssub = nc.declare_dram_parameter("slot_sub", [P, NT], I32, isOutput=False)
    t_valid = nc.declare_dram_parameter("valid", [P, NT], F32, isOutput=False)
    t_oh = nc.declare_dram_parameter("ohrel", [N_REL + 1, S], BF16, isOutput=False)
    t_rela = nc.declare_dram_parameter("rela", [N_REL, DIM], F32, isOutput=False)
    t_wstat = nc.declare_dram_parameter("wstat", [P, 4 * 384], F32, isOutput=False)
    t_w13 = nc.declare_dram_parameter("w13", [P, DIM], F32, isOutput=False)
    t_bagg = nc.declare_dram_parameter("bagg", [1, DIM], F32, isOutput=False)
    t_ws1 = nc.declare_dram_parameter("ws1rep", [8, DIM], F32, isOutput=False)
    t_ws2 = nc.declare_dram_parameter("ws2rep", [P, DIM], F32, isOutput=False)
    t_bsc = nc.declare_dram_parameter("bscore", [P, 1], F32, isOutput=False)
    t_deg = nc.declare_dram_parameter("degb", [P, NB], F32, isOutput=False)
    t_corr = nc.declare_dram_parameter("corrb", [P, NB], F32, isOutput=False)
    t_hpi = nc.declare_dram_parameter("hprev_idx", [P, NB], I32, isOutput=False)
    t_hpm = nc.declare_dram_parameter("hprev_msk", [P, NB], F32, isOutput=False)
    t_nboh = nc.declare_dram_parameter("nboh", [8, NB * P], BF16, isOutput=False)
    t_uoh = nc.declare_dram_parameter("uoh", [P, NB * 8], BF16, isOutput=False)
    t_out = nc.declare_dram_parameter("out", [NB * P, DIM], F32, isOutput=True)

    d_hu_in = nc.dram_tensor("hu_in", [8, DIM], F32)
    d_hu_out = nc.dram_tensor("hu_out", [8, DIM], F32)

    with TileContext(nc) as tc:
        with tc.tile_pool(name="const", bufs=1) as cp, \
             tc.tile_pool(name="stats", bufs=1) as stp, \
             tc.tile_pool(name="ug", bufs=2) as ugp, \
             tc.tile_pool(name="u16", bufs=2) as u16p, \
             tc.tile_pool(name="ohp", bufs=2) as ohp, \
             tc.tile_pool(name="chain", bufs=2) as chp, \
             tc.tile_pool(name="msgb", bufs=2) as msgp, \
             tc.tile_pool(name="work", bufs=6) as wp, \
             tc.tile_pool(name="prolog", bufs=1) as plp, \
             tc.tile_pool(name="epi", bufs=1) as ep, \
             tc.tile_pool(name="psB", bufs=4, space="PSUM") as psB, \
             tc.tile_pool(name="psW", bufs=2, space="PSUM") as psW, \
             tc.tile_pool(name="psE", bufs=1, space="PSUM") as psE:

            # ---------------- constants / prologue ----------------
            ident = cp.tile([P, P], BF16)
            make_identity(nc, ident[:])
            ones1 = cp.tile([1, P], BF16)
            nc.vector.memset(ones1[:], 1.0)

            ssub_sb = cp.tile([P, NT], I32)
            nc.sync.dma_start(out=ssub_sb[:], in_=t_ssub[:, :])
            valid_sb = cp.tile([P, NT], F32)
            nc.sync.dma_start(out=valid_sb[:], in_=t_valid[:, :])

            # weights -> bf16
            wstat16 = cp.tile([P, 4 * 384], BF16)
            wsf = plp.tile([P, 4 * 384], F32, tag="wsf")
            nc.sync.dma_start(out=wsf[:], in_=t_wstat[:, :])
            nc.vector.tensor_copy(out=wstat16[:], in_=wsf[:])
            w13_16 = cp.tile([P, DIM], BF16)
            w13f = plp.tile([P, DIM], F32, tag="w13f")
            nc.sync.dma_start(out=w13f[:], in_=t_w13[:, :])
            nc.vector.tensor_copy(out=w13_16[:], in_=w13f[:])
            bagg16 = cp.tile([1, DIM], BF16)
            baggf = plp.tile([1, DIM], F32, tag="bgf")
            nc.sync.dma_start(out=baggf[:], in_=t_bagg[:, :])
            nc.vector.tensor_copy(out=bagg16[:], in_=baggf[:])
            ws1_16 = cp.tile([8, DIM], BF16)
            ws1f = plp.tile([8, DIM], F32, tag="ws1f")
            nc.sync.dma_start(out=ws1f[:], in_=t_ws1[:, :])
            nc.vector.tensor_copy(out=ws1_16[:], in_=ws1f[:])
            ws2_16 = cp.tile([P, DIM], BF16)
            ws2f = plp.tile([P, DIM], F32, tag="ws2f")
            nc.sync.dma_start(out=ws2f[:], in_=t_ws2[:, :])
            nc.vector.tensor_copy(out=ws2_16[:], in_=ws2f[:])
            bsc_sb = cp.tile([P, 1], F32)
            nc.sync.dma_start(out=bsc_sb[:], in_=t_bsc[:, :])

            # brhs [44, 129]: rows 0..42 = [ExpR | y2], row 43 = zeros
            relaf = cp.tile([N_REL, DIM], F32)
            nc.sync.dma_start(out=relaf[:], in_=t_rela[:, :])
            brhs = cp.tile([N_REL + 1, DIM + 1], BF16)
            nc.vector.memset(brhs[:], 0.0)
            rsc = cp.tile([N_REL, 8], F32)
            scr43 = plp.tile([N_REL, DIM], BF16, tag="scr43")
            nc.vector.scalar_tensor_tensor(out=scr43[:], in0=relaf[:], scalar=1.0,
                                           in1=relaf[:], op0=ALU.mult, op1=ALU.mult,
                                           accum_out=rsc[:, 0:1])
            # sA = 1 + z*(-1/3 + z*(2/15 - z*17/315)), z = nsq
            z = rsc[:, 0:1]
            nc.vector.tensor_scalar(out=rsc[:, 1:2], in0=z, scalar1=-17.0 / 315.0,
                                    scalar2=2.0 / 15.0, op0=ALU.mult, op1=ALU.add)
            nc.vector.tensor_tensor(out=rsc[:, 1:2], in0=rsc[:, 1:2], in1=z, op=ALU.mult)
            nc.vector.tensor_scalar(out=rsc[:, 1:2], in0=rsc[:, 1:2], scalar1=-1.0 / 3.0,
                                    scalar2=None, op0=ALU.add)
            nc.vector.tensor_tensor(out=rsc[:, 1:2], in0=rsc[:, 1:2], in1=z, op=ALU.mult)
            nc.vector.tensor_scalar(out=rsc[:, 1:2], in0=rsc[:, 1:2], scalar1=1.0,
                                    scalar2=None, op0=ALU.add)          # sA_r
            nc.vector.tensor_scalar(out=brhs[0:N_REL, 0:DIM], in0=relaf[:],
                                    scalar1=rsc[:, 1:2], scalar2=None, op0=ALU.mult)
            scr43b = plp.tile([N_REL, DIM], BF16, tag="scr43b")
            nc.vector.scalar_tensor_tensor(out=scr43b[:], in0=brhs[0:N_REL, 0:DIM],
                                           scalar=1.0, in1=brhs[0:N_REL, 0:DIM],
                                           op0=ALU.mult, op1=ALU.mult,
                                           accum_out=rsc[:, 2:3])
            nc.vector.tensor_copy(out=brhs[0:N_REL, DIM:DIM + 1], in_=rsc[:, 2:3])

            # deg-derived per-node scalars
            degb = cp.tile([P, NB], F32)
            nc.sync.dma_start(out=degb[:], in_=t_deg[:, :])
            corrb = cp.tile([P, NB], F32)
            nc.sync.dma_start(out=corrb[:], in_=t_corr[:, :])
            hpm_sb = cp.tile([P, NB], F32)
            nc.sync.dma_start(out=hpm_sb[:], in_=t_hpm[:, :])
            hpi_sb = cp.tile([P, NB], I32)
            nc.sync.dma_start(out=hpi_sb[:], in_=t_hpi[:, :])
            invdeg = cp.tile([P, NB], F32)
            nc.vector.tensor_scalar(out=invdeg[:], in0=degb[:], scalar1=1.0,
                                    scalar2=None, op0=ALU.max)
            nc.vector.reciprocal(out=invdeg[:], in_=invdeg[:])
            amp = cp.tile([P, NB], F32)
            nc.scalar.activation(out=amp[:], in_=degb[:], func=AF.Ln, bias=1.0)
            att = cp.tile([P, NB], F32)
            nc.vector.tensor_scalar(out=att[:], in0=amp[:], scalar1=1e-5,
                                    scalar2=None, op0=ALU.max)
            nc.vector.reciprocal(out=att[:], in_=att[:])

            nboh_sb = cp.tile([8, NB * P], BF16)
            nc.sync.dma_start(out=nboh_sb[:], in_=t_nboh[:, :])
            uoh_sb = cp.tile([P, NB * 8], BF16)
            nc.sync.dma_start(out=uoh_sb[:], in_=t_uoh[:, :])

            htil = stp.tile([P, NB * DIM], BF16)
            t2col = stp.tile([P, NB], F32)
            pre_col = stp.tile([P, NB], F32)
            pu_acc = stp.tile([8, DIM], F32)
            nc.vector.memset(pu_acc[:], 0.0)

            # ---------------- main loop ----------------
            state = {}

            def batch_epilogue(j, mbuf, L):
                mview = mbuf[:].rearrange("p (l c) -> p l c", c=2 * DIM)
                # trees: max / min over the L msg blocks
                if L == 1:
                    mx16 = mbuf[:, 0:DIM]
                    mn16 = mbuf[:, 0:DIM]
                else:
                    smx = ep.tile([P, ((LMAX + 1) // 2) * DIM], BF16, tag="smx")
                    smn = ep.tile([P, ((LMAX + 1) // 2) * DIM], BF16, tag="smn")
                    sxv = smx[:].rearrange("p (l c) -> p l c", c=DIM)
                    snv = smn[:].rearrange("p (l c) -> p l c", c=DIM)
                    h2 = (L + 1) // 2
                    nc.vector.tensor_tensor(
                        out=sxv[:, 0:h2, :], in0=mview[:, 0:h2, 0:DIM],
                        in1=mview[:, L - h2:L, 0:DIM], op=ALU.max)
                    nc.vector.tensor_tensor(
                        out=snv[:, 0:h2, :], in0=mview[:, 0:h2, 0:DIM],
                        in1=mview[:, L - h2:L, 0:DIM], op=ALU.min)
                    m = h2
                    while m > 1:
                        h2 = (m + 1) // 2
                        nc.vector.tensor_tensor(
                            out=sxv[:, 0:h2, :], in0=sxv[:, 0:h2, :],
                            in1=sxv[:, m - h2:m, :], op=ALU.max)
                        nc.vector.tensor_tensor(
                            out=snv[:, 0:h2, :], in0=snv[:, 0:h2, :],
                            in1=snv[:, m - h2:m, :], op=ALU.min)
                        m = h2
                    mx16 = smx[:, 0:DIM]
                    mn16 = smn[:, 0:DIM]
                # corrected sums (padding duplicated edge 0)
                pw = state["psW"][j]
                e0 = ep.tile([P, DIM], F32, tag="e0")
                nc.vector.scalar_tensor_tensor(
                    out=e0[:], in0=mbuf[:, 0:DIM], scalar=corrb[:, j:j + 1],
                    in1=pw[:, 0:DIM], op0=ALU.mult, op1=ALU.add)
                e1 = ep.tile([P, DIM], F32, tag="e1")
                nc.vector.scalar_tensor_tensor(
                    out=e1[:], in0=mbuf[:, DIM:2 * DIM], scalar=corrb[:, j:j + 1],
                    in1=pw[:, DIM:2 * DIM], op0=ALU.mult, op1=ALU.add)
                mean16 = ep.tile([P, DIM], BF16, tag="mean16")
                nc.vector.tensor_scalar(out=mean16[:], in0=e0[:],
                                        scalar1=invdeg[:, j:j + 1], scalar2=None,
                                        op0=ALU.mult)
                ms = ep.tile([P, DIM], F32, tag="ms")
                nc.vector.tensor_tensor(out=ms[:], in0=mean16[:], in1=mean16[:],
                                        op=ALU.mult)
                var = ep.tile([P, DIM], F32, tag="var")
                nc.vector.scalar_tensor_tensor(
                    out=var[:], in0=e1[:], scalar=invdeg[:, j:j + 1], in1=ms[:],
                    op0=ALU.mult, op1=ALU.subtract)
                nc.vector.tensor_scalar(out=var[:], in0=var[:], scalar1=0.0,
                                        scalar2=1e-10, op0=ALU.max, op1=ALU.add)
                std16 = ep.tile([P, DIM], BF16, tag="std16")
                nc.scalar.activation(out=std16[:], in_=var[:], func=AF.Sqrt)
                # hprev
                hpf = ep.tile([P, DIM], F32, tag="hpf")
                nc.gpsimd.indirect_dma_start(
                    out=hpf[:], out_offset=None, in_=t_hid[:, :],
                    in_offset=bass.IndirectOffsetOnAxis(ap=hpi_sb[:, j:j + 1], axis=0))
                hp16 = ep.tile([P, DIM], BF16, tag="hp16")
                nc.vector.tensor_scalar(out=hp16[:], in0=hpf[:],
                                        scalar1=hpm_sb[:, j:j + 1], scalar2=None,
                                        op0=ALU.mult)
                # transposes -> feature-major
                statT = ep.tile([P, 5 * DIM], BF16, tag="statT")
                for k, x in enumerate([mean16[:], mx16, mn16, std16[:], hp16[:]]):
                    tp = psE.tile([P, DIM], BF16, tag="misc")
                    nc.tensor.transpose(out=tp[:], in_=x, identity=ident[:])
                    nc.scalar.copy(out=statT[:, k * DIM:(k + 1) * DIM], in_=tp[:])
                # W matmuls: pc3 [128 nodes, 384] = [P1 | P2 | P3]
                pc3 = psE.tile([P, 384], F32, tag="pc3")
                for k in range(4):
                    nc.tensor.matmul(out=pc3[:, :], lhsT=statT[:, k * DIM:(k + 1) * DIM],
                                     rhs=wstat16[:, k * 384:(k + 1) * 384],
                                     start=(k == 0), stop=False)
                nc.tensor.matmul(out=pc3[:, 0:DIM], lhsT=statT[:, 4 * DIM:5 * DIM],
                                 rhs=w13_16[:], start=False, stop=False)
                nc.tensor.matmul(out=pc3[:, 0:DIM], lhsT=ones1[:, :],
                                 rhs=bagg16[:], start=False, stop=True)
                p1s = ep.tile([P, DIM], F32, tag="p1s")
                nc.scalar.copy(out=p1s[:], in_=pc3[:, 0:DIM])
                h1 = ep.tile([P, DIM], F32, tag="h1")
                nc.vector.scalar_tensor_tensor(
                    out=h1[:], in0=pc3[:, DIM:2 * DIM], scalar=amp[:, j:j + 1],
                    in1=p1s[:], op0=ALU.mult, op1=ALU.add)
                hts = htil[:, j * DIM:(j + 1) * DIM]
                nc.vector.scalar_tensor_tensor(
                    out=hts, in0=pc3[:, 2 * DIM:3 * DIM], scalar=att[:, j:j + 1],
                    in1=h1[:], op0=ALU.mult, op1=ALU.add)
                # t2 accumulation and user-row matmul
                scrt = ep.tile([P, DIM], BF16, tag="scrt")
                nc.vector.scalar_tensor_tensor(
                    out=scrt[:], in0=hts, scalar=1.0, in1=ws2_16[:],
                    op0=ALU.mult, op1=ALU.mult, accum_out=t2col[:, j:j + 1])
                pu = psE.tile([8, DIM], F32, tag="misc")
                nc.tensor.matmul(out=pu[:], lhsT=uoh_sb[:, j * 8:(j + 1) * 8],
                                 rhs=hts, start=True, stop=True)
                nc.vector.tensor_tensor(out=pu_acc[:], in0=pu_acc[:], in1=pu[:],
                                        op=ALU.add)

            for sg0 in range(0, NT, SG):
                sgn = min(SG, NT - sg0)
                ug = ugp.tile([P, SG * DIM], F32, tag="ug")
                for ti in range(sgn):
                    nc.gpsimd.indirect_dma_start(
                        out=ug[:, ti * DIM:(ti + 1) * DIM], out_offset=None,
                        in_=t_hid[:, :],
                        in_offset=bass.IndirectOffsetOnAxis(
                            ap=ssub_sb[:, sg0 + ti:sg0 + ti + 1], axis=0))
                u16 = u16p.tile([P, SG * DIM], BF16, tag="u16")
                for ci in range(0, sgn, 4):
                    cn4 = min(4, sgn - ci)
                    nc.scalar.copy(out=u16[:, ci * DIM:(ci + cn4) * DIM],
                                   in_=ug[:, ci * DIM:(ci + cn4) * DIM])
                ohsg = ohp.tile([N_REL + 1, SG * P], BF16, tag="ohsg")
                nc.sync.dma_start(out=ohsg[:, 0:sgn * P],
                                  in_=t_oh[:, sg0 * P:(sg0 + sgn) * P])
                RW = DIM + 1
                b16f = chp.tile([P, SG * RW], BF16, tag="b16sg")
                stage = chp.tile([P, 3 * SG], F32, tag="stage")
                for ti in range(sgn):
                    # one psum tile per B-matmul (single writer, single reader)
                    pb = psB.tile([P, RW], F32, tag="pb")
                    nc.tensor.matmul(out=pb[:],
                                     lhsT=ohsg[:, ti * P:(ti + 1) * P],
                                     rhs=brhs[:], start=True, stop=True)
                    nc.scalar.copy(out=b16f[:, ti * RW:(ti + 1) * RW], in_=pb[:])
                y2v = b16f[:].rearrange("p (t c) -> p t c", c=RW)[:, 0:sgn, DIM:DIM + 1]
                nc.vector.tensor_copy(out=stage[:, 2 * SG:2 * SG + sgn], in_=y2v)
                for ti in range(sgn):
                    u1 = u16[:, ti * DIM:(ti + 1) * DIM]
                    b1 = b16f[:, ti * RW:ti * RW + DIM]
                    scr = wp.tile([P, DIM], BF16, tag="scr")
                    nc.vector.scalar_tensor_tensor(
                        out=scr[:], in0=u1, scalar=1.0, in1=u1,
                        op0=ALU.mult, op1=ALU.mult, accum_out=stage[:, ti:ti + 1])
                    nc.vector.scalar_tensor_tensor(
                        out=scr[:], in0=u1, scalar=1.0, in1=b1,
                        op0=ALU.mult, op1=ALU.mult,
                        accum_out=stage[:, SG + ti:SG + ti + 1])

                # ---- chain on [P, sgn] f32 ----
                C = chp.tile([P, 10 * SG], F32, tag="chainC")
                def cc_(i):
                    return C[:, i * SG:i * SG + sgn]
                nsq = stage[:, 0:sgn]
                xyu = stage[:, SG:SG + sgn]
                y2 = stage[:, 2 * SG:2 * SG + sgn]
                # sA poly
                nc.vector.tensor_scalar(out=cc_(0), in0=nsq, scalar1=-17.0 / 315.0,
                                        scalar2=2.0 / 15.0, op0=ALU.mult, op1=ALU.add)
                nc.vector.tensor_tensor(out=cc_(0), in0=cc_(0), in1=nsq, op=ALU.mult)
                nc.vector.tensor_scalar(out=cc_(0), in0=cc_(0), scalar1=-1.0 / 3.0,
                                        scalar2=None, op0=ALU.add)
                nc.vector.tensor_tensor(out=cc_(0), in0=cc_(0), in1=nsq, op=ALU.mult)
                nc.vector.tensor_scalar(out=cc_(0), in0=cc_(0), scalar1=1.0,
                                        scalar2=None, op0=ALU.add)        # sA
                nc.vector.tensor_tensor(out=cc_(1), in0=cc_(0), in1=xyu, op=ALU.mult)  # xy
                nc.vector.tensor_tensor(out=cc_(2), in0=cc_(0), in1=cc_(0), op=ALU.mult)
                nc.vector.tensor_tensor(out=cc_(2), in0=cc_(2), in1=nsq, op=ALU.mult)  # x2
                nc.vector.tensor_tensor(out=cc_(3), in0=cc_(2), in1=y2, op=ALU.mult)
                nc.vector.scalar_tensor_tensor(out=cc_(3), in0=cc_(1), scalar=2.0,
                                               in1=cc_(3), op0=ALU.mult, op1=ALU.add)
                nc.vector.tensor_scalar(out=cc_(3), in0=cc_(3), scalar1=1.0,
                                        scalar2=None, op0=ALU.add)        # den
                nc.vector.reciprocal(out=cc_(3), in_=cc_(3))              # rden
                nc.vector.tensor_scalar(out=cc_(4), in0=y2, scalar1=1.0,
                                        scalar2=None, op0=ALU.add)
                nc.vector.scalar_tensor_tensor(out=cc_(4), in0=cc_(1), scalar=2.0,
                                               in1=cc_(4), op0=ALU.mult, op1=ALU.add)
                nc.vector.tensor_tensor(out=cc_(4), in0=cc_(4), in1=cc_(3), op=ALU.mult)  # cA
                nc.vector.tensor_scalar(out=cc_(5), in0=cc_(2), scalar1=-1.0,
                                        scalar2=1.0, op0=ALU.mult, op1=ALU.add)
                nc.vector.tensor_tensor(out=cc_(5), in0=cc_(5), in1=cc_(3), op=ALU.mult)  # cB
                # m2 = cA^2 x2 + 2 cA cB xy + cB^2 y2
                nc.vector.tensor_tensor(out=cc_(6), in0=cc_(4), in1=cc_(4), op=ALU.mult)
                nc.vector.tensor_tensor(out=cc_(6), in0=cc_(6), in1=cc_(2), op=ALU.mult)
                nc.vector.tensor_tensor(out=cc_(7), in0=cc_(4), in1=cc_(5), op=ALU.mult)
                nc.vector.tensor_tensor(out=cc_(7), in0=cc_(7), in1=cc_(1), op=ALU.mult)
                nc.vector.scalar_tensor_tensor(out=cc_(6), in0=cc_(7), scalar=2.0,
                                               in1=cc_(6), op0=ALU.mult, op1=ALU.add)
                nc.vector.tensor_tensor(out=cc_(7), in0=cc_(5), in1=cc_(5), op=ALU.mult)
                nc.vector.tensor_tensor(out=cc_(7), in0=cc_(7), in1=y2, op=ALU.mult)
                nc.vector.tensor_tensor(out=cc_(6), in0=cc_(6), in1=cc_(7), op=ALU.add)  # m2
                nc.vector.tensor_scalar(out=cc_(6), in0=cc_(6), scalar1=0.0,
                                        scalar2=None, op0=ALU.max)
                # cc poly: 1 + w(1/3 + w(1/5 + w(1/7 + w(1/9 + w/11))))
                w_ = cc_(6)
                nc.vector.tensor_scalar(out=cc_(7), in0=w_, scalar1=1.0 / 11.0,
                                        scalar2=1.0 / 9.0, op0=ALU.mult, op1=ALU.add)
                for coef in (1.0 / 7.0, 1.0 / 5.0, 1.0 / 3.0):
                    nc.vector.tensor_tensor(out=cc_(7), in0=cc_(7), in1=w_, op=ALU.mult)
                    nc.vector.tensor_scalar(out=cc_(7), in0=cc_(7), scalar1=coef,
                                            scalar2=None, op0=ALU.add)
                nc.vector.tensor_tensor(out=cc_(7), in0=cc_(7), in1=w_, op=ALU.mult)
                nc.vector.tensor_scalar(out=cc_(7), in0=cc_(7), scalar1=1.0,
                                        scalar2=None, op0=ALU.add)        # cc
                vsl = valid_sb[:, sg0:sg0 + sgn]
                fac = chp.tile([P, 2 * SG], F32, tag="fac")
                nc.vector.tensor_tensor(out=cc_(8), in0=cc_(4), in1=cc_(7), op=ALU.mult)
                nc.vector.tensor_tensor(out=cc_(8), in0=cc_(8), in1=cc_(0), op=ALU.mult)
                nc.vector.tensor_tensor(out=fac[:, 0:sgn], in0=cc_(8), in1=vsl,
                                        op=ALU.mult)                      # fA
                nc.vector.tensor_tensor(out=cc_(9), in0=cc_(5), in1=cc_(7), op=ALU.mult)
                nc.vector.tensor_tensor(out=fac[:, SG:SG + sgn], in0=cc_(9), in1=vsl,
                                        op=ALU.mult)                      # fB

                # ---- per tile: msg, msgsq, sums; epilogue at batch end ----
                for ti in range(sgn):
                    t = sg0 + ti
                    j = int(t2b[t])
                    pos = int(t2pos[t])
                    L = int(Lb[j])
                    if pos == 0:
                        mbuf = msgp.tile([P, LMAX * 2 * DIM], BF16, tag="mbuf")
                        state["mbuf"] = mbuf
                        pwj = psW.tile([P, 2 * DIM], F32, tag="pw")
                        state.setdefault("psW", {})[j] = pwj
                    mbuf = state["mbuf"]
                    pwj = state["psW"][j]
                    u1 = u16[:, ti * DIM:(ti + 1) * DIM]
                    b1 = b16f[:, ti * RW:ti * RW + DIM]
                    t1 = wp.tile([P, DIM], BF16, tag="t1")
                    nc.vector.tensor_scalar(out=t1[:], in0=u1,
                                            scalar1=fac[:, ti:ti + 1], scalar2=None,
                                            op0=ALU.mult)
                    mslice = mbuf[:, pos * 2 * DIM:pos * 2 * DIM + DIM]
                    nc.vector.scalar_tensor_tensor(
                        out=mslice, in0=b1, scalar=fac[:, SG + ti:SG + ti + 1],
                        in1=t1[:], op0=ALU.mult, op1=ALU.add)
                    nc.scalar.activation(
                        out=mbuf[:, pos * 2 * DIM + DIM:(pos + 1) * 2 * DIM],
                        in_=mslice, func=AF.Square)
                    nc.tensor.matmul(out=pwj[:, :], lhsT=ident[:],
                                     rhs=mbuf[:, pos * 2 * DIM:(pos + 1) * 2 * DIM],
                                     start=(pos == 0), stop=(pos == L - 1))
                    if pos == L - 1:
                        batch_epilogue(j, mbuf, L)
                        del state["psW"][j]

            # ---------------- allreduce + gate + output ----------------
            nc.sync.dma_start(out=d_hu_in[:, :], in_=pu_acc[:])
            import concourse.mybir as mybir2
            nc.gpsimd.collective_compute(
                "AllReduce", mybir2.AluOpType.add,
                replica_groups=[list(range(NCORES))],
                ins=[d_hu_in[:, :]], outs=[d_hu_out[:, :]])
            hu2 = wp.tile([8, DIM], F32, tag="hu2")
            nc.sync.dma_start(out=hu2[:], in_=d_hu_out[:, :])
            hu16 = cp.tile([8, DIM], BF16)
            nc.vector.tensor_copy(out=hu16[:], in_=hu2[:])
            su = cp.tile([8, 1], F32)
            scr8 = wp.tile([8, DIM], BF16, tag="scr8")
            nc.vector.scalar_tensor_tensor(out=scr8[:], in0=hu16[:], scalar=1.0,
                                           in1=ws1_16[:], op0=ALU.mult, op1=ALU.mult,
                                           accum_out=su[:, 0:1])
            su16 = cp.tile([8, 1], BF16)
            nc.vector.tensor_copy(out=su16[:], in_=su[:])
            for j in range(NB):
                psu = psE.tile([P, 1], F32, tag="misc")
                nc.tensor.matmul(out=psu[:], lhsT=nboh_sb[:, j * P:(j + 1) * P],
                                 rhs=su16[:], start=True, stop=True)
                nc.vector.tensor_tensor(out=pre_col[:, j:j + 1], in0=t2col[:, j:j + 1],
                                        in1=psu[:], op=ALU.add)
            alpha = cp.tile([P, NB], F32)
            nc.scalar.activation(out=alpha[:], in_=pre_col[:], func=AF.Sigmoid,
                                 bias=bsc_sb[:, 0:1])
            for j in range(NB):
                ob = wp.tile([P, DIM], F32, tag="ob")
                nc.vector.tensor_scalar(out=ob[:], in0=htil[:, j * DIM:(j + 1) * DIM],
                                        scalar1=alpha[:, j:j + 1], scalar2=None,
                                        op0=ALU.mult)
                nc.sync.dma_start(out=t_out[j * P:(j + 1) * P, :], in_=ob[:])
    return nc


# ----------------------------------------------------------------------------
# entry point
# ----------------------------------------------------------------------------

def kernel(hidden, rela_embed, W_agg, b_agg, W_score, b_score,
           edges, nodes, q_sub, old_nodes_new_idx):
    import ml_dtypes
    from concourse.bass_utils import run_bass_kernel_spmd

    struct, per_core, inv, user_idx, nb_arr = preprocess(
        edges, nodes, q_sub, old_nodes_new_idx)
    NB, NT, S = struct["NB"], struct["NT"], struct["S"]

    nc = build_graph(struct)

    hidden = np.ascontiguousarray(np.asarray(hidden, np.float32))
    rela = np.ascontiguousarray(np.asarray(rela_embed, np.float32))
    W = np.asarray(W_agg, np.float32)
    # wstat[:, k*384:(k+1)*384] = [W1_k | W2_k | W3_k] for stat k
    wstat = np.zeros((P, 4 * 384), np.float32)
    for k in range(4):
        wstat[:, k * 384 + 0:k * 384 + 128] = W[k * DIM:(k + 1) * DIM]
        wstat[:, k * 384 + 128:k * 384 + 256] = W[(4 + k) * DIM:(5 + k) * DIM]
        wstat[:, k * 384 + 256:k * 384 + 384] = W[(8 + k) * DIM:(9 + k) * DIM]
    w13 = np.ascontiguousarray(W[12 * DIM:13 * DIM])
    bagg = np.asarray(b_agg, np.float32).reshape(1, DIM)
    ws = np.asarray(W_score, np.float32)
    ws1rep = np.repeat(ws[0:DIM, 0][None, :], 8, axis=0)
    ws2rep = np.repeat(ws[DIM:2 * DIM, 0][None, :], P, axis=0)
    bscore_col = np.full((P, 1), np.asarray(b_score, np.float32)[0], np.float32)

    in_maps = []
    for c in range(NCORES):
        pc = per_core[c]
        ohrel = np.zeros((N_REL + 1, S), dtype=ml_dtypes.bfloat16)
        sr = pc["srel"].T.reshape(-1)          # slot order: tile-major, partition fast
        # slot s of tile t lives at column t*P+p ; srel is [P, NT]
        cols = np.arange(S)
        ohrel[sr.reshape(NT, P).reshape(-1), cols] = 1.0
        nboh = np.zeros((8, NB * P), dtype=ml_dtypes.bfloat16)
        uoh = np.zeros((P, NB * 8), dtype=ml_dtypes.bfloat16)
        nid = pc["node_id"]
        ok = nid >= 0
        nboh[nb_arr[nid[ok]], np.where(ok)[0]] = 1.0
        for b in range(BATCH):
            wpos = np.where(nid == user_idx[b])[0]
            if len(wpos):
                n = int(wpos[0])
                uoh[n % P, (n // P) * 8 + b] = 1.0
        in_maps.append({
            "hidden": hidden, "slot_sub": pc["ssub"], "valid": pc["valid"],
            "ohrel": np.asarray(ohrel), "rela": rela, "wstat": wstat,
            "w13": w13, "bagg": bagg, "ws1rep": ws1rep, "ws2rep": ws2rep,
            "bscore": bscore_col, "degb": pc["deg"], "corrb": pc["corr"],
            "hprev_idx": pc["hpidx"], "hprev_msk": pc["hpmask"],
            "nboh": np.asarray(nboh), "uoh": np.asarray(uoh),
        })

    do_trace = bool(int(os.environ.get("KERNEL_TRACE", "0")))
    if do_trace:
        _ensure_ntff_hook()
    res = run_bass_kernel_spmd(nc, in_maps, core_ids=list(range(NCORES)),
                               trace=do_trace,
                               tmpdir=os.environ.get("KERNEL_TRACE_DIR"))
    kernel.last_exec_time_ns = res.exec_time_ns

    out = np.zeros((N_NODES, DIM), dtype=np.float32)
    for c in range(NCORES):
        oc = res.results[c]["out"]
        nid = per_core[c]["node_id"]
        ok = nid >= 0
        out[nid[ok]] = oc[ok]
    return out


# revision 23
# speedup vs baseline: 1.1404x; 1.1404x over previous
"""Trainium2 Bass kernel for nn_AdaptiveSubgraphLayer (hyperbolic GNN + PNA).

v3 node-parallel layout (8 NeuronCores, SPMD):
  - Host (int-only): per core, nodes sorted by degree desc, batches of 128
    nodes (node -> partition). Batch j owns Lb[j] tile columns; tile l holds
    edge l of each of the 128 nodes (padding duplicates edge 0, corrected in
    the sums; deg-0/fake nodes hit the zero rel-row with valid=0 -> msg 0).
    Batch structure identical across cores.
  - Device per tile: indirect-gather hidden rows (slot-major), one-hot
    matmul expands ExpR[rel] (+|y|^2 col) into PSUM-bf16. All hyperbolic
    maps collapse to polynomials in (|u|^2, u.B, |B|^2) -- tanh and atanh
    are replaced by odd-series in the squared norms (valid: |m| < 0.6),
    so no activation tables in the hot loop. msg = fA*u + fB*B.
  - Segment sums via identity-matmul PSUM accumulation over the batch's
    tiles; max/min via pairwise-tree tensor_tensor over the batch buffer.
  - Batch epilogue: PNA mean/std/max/min node-major, 5 PE transposes,
    W_agg matmuls (384-wide rhs), scaler fold; h_tilde stays in SBUF.
  - Tiny allreduce shares the 8 user rows; sigmoid gate at the end.
"""
import sys
import os

sys.path.insert(0, "/opt/trn_rl_repo")
sys.path.insert(0, os.path.dirname(os.path.abspath(__file__)))

import numpy as np

N_NODES, N_PREV, N_EDGES, DIM, BATCH = 100000, 80000, 1000000, 128, 8
N_REL = 43
NCORES = 8
P = 128
SG = 14
MIN_NORM = 1e-15


# --- inlined walrus single-wait workaround (kernel.py must be self-contained) ---
_TILE_PATCH_SRC = '"""Workaround: the walrus build in this container supports only ONE sem-wait\nper ISA instruction; Tile\'s scheduler attaches several. After TileContext\nlowering (including the tail drain/barrier), sweep every basic block and move\nexcess waits onto same-engine nop instructions inserted immediately before\nthe over-subscribed instruction."""\nimport concourse.mybir as mybir\nfrom concourse.tile import TileContext\n\nMAX_WAITS = 1\nCOMPUTE_MAX_WAITS = 1\n_CTRL = ("InstNoOp", "InstDrain", "InstEventSemOp")\n\n\ndef _limit(inst):\n    return MAX_WAITS if type(inst).__name__ in _CTRL else COMPUTE_MAX_WAITS\n\n_orig_drain = TileContext._drain_and_barrier\n\n\ndef _split_all_waits(nc):\n    for bb in nc.main_func.blocks:\n        insts = list(bb.instructions)\n        need = []\n        for inst in insts:\n            si = inst.sync_info\n            if si is not None and len(si.on_wait) > _limit(inst):\n                need.append(inst)\n        if not need:\n            continue\n        patch = {}\n        created = []\n        for inst in need:\n            si = inst.sync_info\n            lim = _limit(inst)\n            waits = list(si.on_wait)\n            si.on_wait = waits[-lim:]\n            rest = waits[:-lim]\n            nops = []\n            eng = nc.engines[inst.engine]\n            for j in range(0, len(rest), MAX_WAITS):\n                nop = eng.nop(nofuse=True)\n                nsi = nop.ins.sync_info\n                if nsi is None:\n                    nop.ins.sync_info = mybir.SyncInfo(\n                        on_wait=rest[j:j + MAX_WAITS], on_update=[])\n                else:\n                    nsi.on_wait = rest[j:j + MAX_WAITS]\n                nops.append(nop.ins)\n                created.append(nop.ins)\n            patch[id(inst)] = nops\n        created_ids = {id(x) for x in created}\n        for bb2 in nc.main_func.blocks:\n            if any(id(x) in created_ids for x in bb2.instructions):\n                bb2.instructions[:] = [\n                    x for x in bb2.instructions if id(x) not in created_ids]\n        out = []\n        for inst in insts:\n            if id(inst) in patch:\n                out.extend(patch[id(inst)])\n            out.append(inst)\n        bb.instructions[:] = out\n\n\ndef _drain_and_barrier(self, tick_clock, wait_clock):\n    _orig_drain(self, tick_clock, wait_clock)\n    _split_all_waits(self.nc)\n\n\ndef install():\n    TileContext._drain_and_barrier = _drain_and_barrier\n'


def _install_tile_patch():
    import types, sys as _sys
    if "tile_patch" in _sys.modules:
        return _sys.modules["tile_patch"]
    m = types.ModuleType("tile_patch")
    exec(_TILE_PATCH_SRC, m.__dict__)
    _sys.modules["tile_patch"] = m
    return m


def _ensure_ntff_hook():
    try:
        import antenv.axon_hooks  # noqa: F401
        return
    except ImportError:
        pass
    import types, sys as _sys
    m = types.ModuleType("antenv.axon_hooks")
    m._hook = None
    def set_axon_ntff_profile_hook(h):
        m._hook = h
    def get_axon_ntff_profile_hook():
        return m._hook
    m.set_axon_ntff_profile_hook = set_axon_ntff_profile_hook
    m.get_axon_ntff_profile_hook = get_axon_ntff_profile_hook
    _sys.modules["antenv.axon_hooks"] = m
    try:
        import antenv
        antenv.axon_hooks = m
    except ImportError:
        pass
    try:
        from trn_agent_boot.trn_boot import _ntff_profile_via_ctypes
        hook = _ntff_profile_via_ctypes("/opt/axon/libaxon_pjrt.so")
        if hook is not None:
            m._hook = hook
    except Exception:
        pass


# ----------------------------------------------------------------------------
# host preprocessing (integers only)
# ----------------------------------------------------------------------------

def preprocess(edges, nodes, q_sub, old_nodes_new_idx):
    sub = np.asarray(edges[:, 4], dtype=np.int64)
    rel = np.asarray(edges[:, 2], dtype=np.int64)
    obj = np.asarray(edges[:, 5], dtype=np.int64)
    deg = np.bincount(obj, minlength=N_NODES)

    cum = np.cumsum(deg)
    bounds = [0] + [int(np.searchsorted(cum, N_EDGES * c / NCORES))
                    for c in range(1, NCORES)] + [N_NODES]

    order = np.argsort(obj, kind="stable")
    sub_s, rel_s = sub[order], rel[order]
    estart = np.zeros(N_NODES + 1, dtype=np.int64)
    estart[1:] = cum

    core_nodes = []
    for c in range(NCORES):
        nid = np.arange(bounds[c], bounds[c + 1])
        o = np.argsort(-deg[nid], kind="stable")
        core_nodes.append(nid[o])

    NB = max((len(n) + P - 1) // P for n in core_nodes)
    Lb = np.ones(NB, dtype=np.int64)
    for c in range(NCORES):
        dd = deg[core_nodes[c]]
        for j in range((len(dd) + P - 1) // P):
            Lb[j] = max(Lb[j], dd[j * P:(j + 1) * P].max(initial=1))
    bstart = np.zeros(NB + 1, dtype=np.int64)
    bstart[1:] = np.cumsum(Lb)
    NT = int(bstart[-1])

    inv = np.full(N_NODES, 2 ** 30, dtype=np.int64)
    inv[np.asarray(old_nodes_new_idx, dtype=np.int64)] = np.arange(N_PREV)

    nb_arr = np.asarray(nodes[:, 0], dtype=np.int64)
    ne_arr = np.asarray(nodes[:, 1], dtype=np.int64)
    user_idx = np.zeros(BATCH, dtype=np.int64)
    for b in range(BATCH):
        m = np.where((nb_arr == b) & (ne_arr == np.asarray(q_sub)[b]))[0]
        user_idx[b] = m[0]

    per_core = []
    for c in range(NCORES):
        cn = core_nodes[c]
        ncn = len(cn)
        ssub = np.zeros((P, NT), dtype=np.int32)
        srel = np.full((P, NT), N_REL, dtype=np.int64)
        valid = np.zeros((P, NT), dtype=np.float32)
        node_id = np.full(NB * P, -1, dtype=np.int64)
        degc = np.zeros((P, NB), dtype=np.float32)
        corr = np.zeros((P, NB), dtype=np.float32)
        hpidx = np.zeros((P, NB), dtype=np.int32)
        hpmask = np.zeros((P, NB), dtype=np.float32)
        for j in range(NB):
            l0, l1 = int(bstart[j]), int(bstart[j + 1])
            L = l1 - l0
            for p in range(P):
                k = j * P + p
                if k >= ncn:
                    continue
                n = int(cn[k])
                node_id[k] = n
                d = int(deg[n])
                degc[p, j] = d
                if inv[n] < N_PREV:
                    hpidx[p, j] = inv[n]
                    hpmask[p, j] = 1.0
                if d == 0:
                    continue
                corr[p, j] = d - L
                e0 = int(estart[n])
                idx = np.arange(L)
                idx[d:] = 0
                ssub[p, l0:l1] = sub_s[e0 + idx]
                srel[p, l0:l1] = rel_s[e0 + idx]
                valid[p, l0:l1] = 1.0
        per_core.append(dict(ssub=ssub, srel=srel, valid=valid,
                             node_id=node_id, deg=degc, corr=corr,
                             hpidx=hpidx, hpmask=hpmask))

    struct = dict(NB=NB, NT=NT, S=NT * P, Lb=Lb, bstart=bstart, bounds=bounds)
    return struct, per_core, inv, user_idx, nb_arr


# ----------------------------------------------------------------------------
# bass graph
# ----------------------------------------------------------------------------

def build_graph(struct):
    import concourse.bass as bass
    import concourse.mybir as mybir
    from concourse.tile import TileContext
    from concourse.masks import make_identity
    _install_tile_patch().install()

    F32, BF16, I32 = mybir.dt.float32, mybir.dt.bfloat16, mybir.dt.int32
    AF = mybir.ActivationFunctionType
    ALU = mybir.AluOpType

    NB, NT, Lb, bstart = struct["NB"], struct["NT"], struct["Lb"], struct["bstart"]
    S = struct["S"]
    LMAX = int(Lb.max())
    # tile -> (batch, pos, is_first, is_last)
    t2b = np.zeros(NT, dtype=np.int64)
    t2pos = np.zeros(NT, dtype=np.int64)
    for j in range(NB):
        t2b[bstart[j]:bstart[j + 1]] = j
        t2pos[bstart[j]:bstart[j + 1]] = np.arange(bstart[j + 1] - bstart[j])

    nc = bass.Bass()
    t_hid = nc.declare_dram_parameter("hidden", [N_PREV, DIM], F32, isOutput=False)
    t_ssub = nc.declare_dram_parameter("slot_sub", [P, NT], I32, isOutput=False)
    t_valid = nc.declare_dram_parameter("valid", [P, NT], F32, isOutput=False)
    t_oh = nc.declare_dram_parameter("ohrel", [N_REL + 1, S], BF16, isOutput=False)
    t_rela = nc.declare_dram_parameter("rela", [N_REL, DIM], F32, isOutput=False)
    t_wstat = nc.declare_dram_parameter("wstat", [P, 4 * 384], F32, isOutput=False)
    t_w13 = nc.declare_dram_parameter("w13", [P, DIM], F32, isOutput=False)
    t_bagg = nc.declare_dram_parameter("bagg", [1, DIM], F32, isOutput=False)
    t_ws1 = nc.declare_dram_parameter("ws1rep", [8, DIM], F32, isOutput=False)
    t_ws2 = nc.declare_dram_parameter("ws2rep", [P, DIM], F32, isOutput=False)
    t_bsc = nc.declare_dram_parameter("bscore", [P, 1], F32, isOutput=False)
    t_deg = nc.declare_dram_parameter("degb", [P, NB], F32, isOutput=False)
    t_corr = nc.declare_dram_parameter("corrb", [P, NB], F32, isOutput=False)
    t_hpi = nc.declare_dram_parameter("hprev_idx", [P, NB], I32, isOutput=False)
    t_hpm = nc.declare_dram_parameter("hprev_msk", [P, NB], F32, isOutput=False)
    t_nboh = nc.declare_dram_parameter("nboh", [8, NB * P], BF16, isOutput=False)
    t_uoh = nc.declare_dram_parameter("uoh", [P, NB * 8], BF16, isOutput=False)
    t_out = nc.declare_dram_parameter("out", [NB * P, DIM], F32, isOutput=True)

    d_hu_in = nc.dram_tensor("hu_in", [8, DIM], F32)
    d_hu_out = nc.dram_tensor("hu_out", [8, DIM], F32)

    with TileContext(nc) as tc:
        with tc.tile_pool(name="const", bufs=1) as cp, \
             tc.tile_pool(name="stats", bufs=1) as stp, \
             tc.tile_pool(name="ug", bufs=3) as ugp, \
             tc.tile_pool(name="u16", bufs=3) as u16p, \
             tc.tile_pool(name="ohp", bufs=3) as ohp, \
             tc.tile_pool(name="chain", bufs=3) as chp, \
             tc.tile_pool(name="msgb", bufs=2) as msgp, \
             tc.tile_pool(name="work", bufs=6) as wp, \
             tc.tile_pool(name="prolog", bufs=1) as plp, \
             tc.tile_pool(name="epi", bufs=2) as ep, \
             tc.tile_pool(name="psB", bufs=4, space="PSUM") as psB, \
             tc.tile_pool(name="psW", bufs=2, space="PSUM") as psW, \
             tc.tile_pool(name="psE", bufs=1, space="PSUM") as psE:

            # ---------------- constants / prologue ----------------
            ident = cp.tile([P, P], BF16)
            make_identity(nc, ident[:])
            ones1 = cp.tile([1, P], BF16)
            nc.vector.memset(ones1[:], 1.0)

            ssub_sb = cp.tile([P, NT], I32)
            nc.sync.dma_start(out=ssub_sb[:], in_=t_ssub[:, :])
            valid_sb = cp.tile([P, NT], F32)
            nc.sync.dma_start(out=valid_sb[:], in_=t_valid[:, :])

            # weights -> bf16
            wstat16 = cp.tile([P, 4 * 384], BF16)
            wsf = plp.tile([P, 4 * 384], F32, tag="wsf")
            nc.sync.dma_start(out=wsf[:], in_=t_wstat[:, :])
            nc.vector.tensor_copy(out=wstat16[:], in_=wsf[:])
            w13_16 = cp.tile([P, DIM], BF16)
            w13f = plp.tile([P, DIM], F32, tag="w13f")
            nc.sync.dma_start(out=w13f[:], in_=t_w13[:, :])
            nc.vector.tensor_copy(out=w13_16[:], in_=w13f[:])
            bagg16 = cp.tile([1, DIM], BF16)
            baggf = plp.tile([1, DIM], F32, tag="bgf")
            nc.sync.dma_start(out=baggf[:], in_=t_bagg[:, :])
            nc.vector.tensor_copy(out=bagg16[:], in_=baggf[:])
            ws1_16 = cp.tile([8, DIM], BF16)
            ws1f = plp.tile([8, DIM], F32, tag="ws1f")
            nc.sync.dma_start(out=ws1f[:], in_=t_ws1[:, :])
            nc.vector.tensor_copy(out=ws1_16[:], in_=ws1f[:])
            ws2_16 = cp.tile([P, DIM], BF16)
            ws2f = plp.tile([P, DIM], F32, tag="ws2f")
            nc.sync.dma_start(out=ws2f[:], in_=t_ws2[:, :])
            nc.vector.tensor_copy(out=ws2_16[:], in_=ws2f[:])
            bsc_sb = cp.tile([P, 1], F32)
            nc.sync.dma_start(out=bsc_sb[:], in_=t_bsc[:, :])

            # brhs [44, 129]: rows 0..42 = [ExpR | y2], row 43 = zeros
            relaf = cp.tile([N_REL, DIM], F32)
            nc.sync.dma_start(out=relaf[:], in_=t_rela[:, :])
            brhs = cp.tile([N_REL + 1, DIM + 1], BF16)
            nc.vector.memset(brhs[:], 0.0)
            rsc = cp.tile([N_REL, 8], F32)
            scr43 = plp.tile([N_REL, DIM], BF16, tag="scr43")
            nc.vector.scalar_tensor_tensor(out=scr43[:], in0=relaf[:], scalar=1.0,
                                           in1=relaf[:], op0=ALU.mult, op1=ALU.mult,
                                           accum_out=rsc[:, 0:1])
            # sA = 1 + z*(-1/3 + z*(2/15 - z*17/315)), z = nsq
            z = rsc[:, 0:1]
            nc.vector.tensor_scalar(out=rsc[:, 1:2], in0=z, scalar1=-17.0 / 315.0,
                                    scalar2=2.0 / 15.0, op0=ALU.mult, op1=ALU.add)
            nc.vector.tensor_tensor(out=rsc[:, 1:2], in0=rsc[:, 1:2], in1=z, op=ALU.mult)
            nc.vector.tensor_scalar(out=rsc[:, 1:2], in0=rsc[:, 1:2], scalar1=-1.0 / 3.0,
                                    scalar2=None, op0=ALU.add)
            nc.vector.tensor_tensor(out=rsc[:, 1:2], in0=rsc[:, 1:2], in1=z, op=ALU.mult)
            nc.vector.tensor_scalar(out=rsc[:, 1:2], in0=rsc[:, 1:2], scalar1=1.0,
                                    scalar2=None, op0=ALU.add)          # sA_r
            nc.vector.tensor_scalar(out=brhs[0:N_REL, 0:DIM], in0=relaf[:],
                                    scalar1=rsc[:, 1:2], scalar2=None, op0=ALU.mult)
            scr43b = plp.tile([N_REL, DIM], BF16, tag="scr43b")
            nc.vector.scalar_tensor_tensor(out=scr43b[:], in0=brhs[0:N_REL, 0:DIM],
                                           scalar=1.0, in1=brhs[0:N_REL, 0:DIM],
                                           op0=ALU.mult, op1=ALU.mult,
                                           accum_out=rsc[:, 2:3])
            nc.vector.tensor_copy(out=brhs[0:N_REL, DIM:DIM + 1], in_=rsc[:, 2:3])

            # deg-derived per-node scalars
            degb = cp.tile([P, NB], F32)
            nc.sync.dma_start(out=degb[:], in_=t_deg[:, :])
            corrb = cp.tile([P, NB], F32)
            nc.sync.dma_start(out=corrb[:], in_=t_corr[:, :])
            hpm_sb = cp.tile([P, NB], F32)
            nc.sync.dma_start(out=hpm_sb[:], in_=t_hpm[:, :])
            hpi_sb = cp.tile([P, NB], I32)
            nc.sync.dma_start(out=hpi_sb[:], in_=t_hpi[:, :])
            invdeg = cp.tile([P, NB], F32)
            nc.vector.tensor_scalar(out=invdeg[:], in0=degb[:], scalar1=1.0,
                                    scalar2=None, op0=ALU.max)
            nc.vector.reciprocal(out=invdeg[:], in_=invdeg[:])
            amp = cp.tile([P, NB], F32)
            nc.scalar.activation(out=amp[:], in_=degb[:], func=AF.Ln, bias=1.0)
            att = cp.tile([P, NB], F32)
            nc.vector.tensor_scalar(out=att[:], in0=amp[:], scalar1=1e-5,
                                    scalar2=None, op0=ALU.max)
            nc.vector.reciprocal(out=att[:], in_=att[:])

            nboh_sb = cp.tile([8, NB * P], BF16)
            nc.sync.dma_start(out=nboh_sb[:], in_=t_nboh[:, :])
            uoh_sb = cp.tile([P, NB * 8], BF16)
            nc.sync.dma_start(out=uoh_sb[:], in_=t_uoh[:, :])

            htil = stp.tile([P, NB * DIM], BF16)
            t2col = stp.tile([P, NB], F32)
            pre_col = stp.tile([P, NB], F32)
            pu_acc = stp.tile([8, DIM], F32)
            nc.vector.memset(pu_acc[:], 0.0)

            # ---------------- main loop ----------------
            state = {}

            def batch_epilogue(j, mbuf, L):
                mview = mbuf[:].rearrange("p (l c) -> p l c", c=2 * DIM)
                # trees: max / min over the L msg blocks
                if L == 1:
                    mx16 = mbuf[:, 0:DIM]
                    mn16 = mbuf[:, 0:DIM]
                else:
                    smx = ep.tile([P, ((LMAX + 1) // 2) * DIM], BF16, tag="smx")
                    smn = ep.tile([P, ((LMAX + 1) // 2) * DIM], BF16, tag="smn")
                    sxv = smx[:].rearrange("p (l c) -> p l c", c=DIM)
                    snv = smn[:].rearrange("p (l c) -> p l c", c=DIM)
                    h2 = (L + 1) // 2
                    nc.vector.tensor_tensor(
                        out=sxv[:, 0:h2, :], in0=mview[:, 0:h2, 0:DIM],
                        in1=mview[:, L - h2:L, 0:DIM], op=ALU.max)
                    nc.vector.tensor_tensor(
                        out=snv[:, 0:h2, :], in0=mview[:, 0:h2, 0:DIM],
                        in1=mview[:, L - h2:L, 0:DIM], op=ALU.min)
                    m = h2
                    while m > 1:
                        h2 = (m + 1) // 2
                        nc.vector.tensor_tensor(
                            out=sxv[:, 0:h2, :], in0=sxv[:, 0:h2, :],
                            in1=sxv[:, m - h2:m, :], op=ALU.max)
                        nc.vector.tensor_tensor(
                            out=snv[:, 0:h2, :], in0=snv[:, 0:h2, :],
                            in1=snv[:, m - h2:m, :], op=ALU.min)
                        m = h2
                    mx16 = smx[:, 0:DIM]
                    mn16 = smn[:, 0:DIM]
                # corrected sums (padding duplicated edge 0)
                pw = state["psW"][j]
                e0 = ep.tile([P, DIM], F32, tag="e0")
                nc.vector.scalar_tensor_tensor(
                    out=e0[:], in0=mbuf[:, 0:DIM], scalar=corrb[:, j:j + 1],
                    in1=pw[:, 0:DIM], op0=ALU.mult, op1=ALU.add)
                e1 = ep.tile([P, DIM], F32, tag="e1")
                nc.vector.scalar_tensor_tensor(
                    out=e1[:], in0=mbuf[:, DIM:2 * DIM], scalar=corrb[:, j:j + 1],
                    in1=pw[:, DIM:2 * DIM], op0=ALU.mult, op1=ALU.add)
                mean16 = ep.tile([P, DIM], BF16, tag="mean16")
                nc.vector.tensor_scalar(out=mean16[:], in0=e0[:],
                                        scalar1=invdeg[:, j:j + 1], scalar2=None,
                                        op0=ALU.mult)
                ms = ep.tile([P, DIM], F32, tag="ms")
                nc.vector.tensor_tensor(out=ms[:], in0=mean16[:], in1=mean16[:],
                                        op=ALU.mult)
                var = ep.tile([P, DIM], F32, tag="var")
                nc.vector.scalar_tensor_tensor(
                    out=var[:], in0=e1[:], scalar=invdeg[:, j:j + 1], in1=ms[:],
                    op0=ALU.mult, op1=ALU.subtract)
                nc.vector.tensor_scalar(out=var[:], in0=var[:], scalar1=0.0,
                                        scalar2=1e-10, op0=ALU.max, op1=ALU.add)
                std16 = ep.tile([P, DIM], BF16, tag="std16")
                nc.scalar.activation(out=std16[:], in_=var[:], func=AF.Sqrt)
                # hprev
                hpf = ep.tile([P, DIM], F32, tag="hpf")
                nc.gpsimd.indirect_dma_start(
                    out=hpf[:], out_offset=None, in_=t_hid[:, :],
                    in_offset=bass.IndirectOffsetOnAxis(ap=hpi_sb[:, j:j + 1], axis=0))
                hp16 = ep.tile([P, DIM], BF16, tag="hp16")
                nc.vector.tensor_scalar(out=hp16[:], in0=hpf[:],
                                        scalar1=hpm_sb[:, j:j + 1], scalar2=None,
                                        op0=ALU.mult)
                # transposes -> feature-major
                statT = ep.tile([P, 5 * DIM], BF16, tag="statT")
                for k, x in enumerate([mean16[:], mx16, mn16, std16[:], hp16[:]]):
                    tp = psE.tile([P, DIM], BF16, tag="misc")
                    nc.tensor.transpose(out=tp[:], in_=x, identity=ident[:])
                    nc.scalar.copy(out=statT[:, k * DIM:(k + 1) * DIM], in_=tp[:])
                # W matmuls: pc3 [128 nodes, 384] = [P1 | P2 | P3]
                pc3 = psE.tile([P, 384], F32, tag="pc3")
                for k in range(4):
                    nc.tensor.matmul(out=pc3[:, :], lhsT=statT[:, k * DIM:(k + 1) * DIM],
                                     rhs=wstat16[:, k * 384:(k + 1) * 384],
                                     start=(k == 0), stop=False)
                nc.tensor.matmul(out=pc3[:, 0:DIM], lhsT=statT[:, 4 * DIM:5 * DIM],
                                 rhs=w13_16[:], start=False, stop=False)
                nc.tensor.matmul(out=pc3[:, 0:DIM], lhsT=ones1[:, :],
                                 rhs=bagg16[:], start=False, stop=True)
                p1s = ep.tile([P, DIM], F32, tag="p1s")
                nc.scalar.copy(out=p1s[:], in_=pc3[:, 0:DIM])
                h1 = ep.tile([P, DIM], F32, tag="h1")
                nc.vector.scalar_tensor_tensor(
                    out=h1[:], in0=pc3[:, DIM:2 * DIM], scalar=amp[:, j:j + 1],
                    in1=p1s[:], op0=ALU.mult, op1=ALU.add)
                hts = htil[:, j * DIM:(j + 1) * DIM]
                nc.vector.scalar_tensor_tensor(
                    out=hts, in0=pc3[:, 2 * DIM:3 * DIM], scalar=att[:, j:j + 1],
                    in1=h1[:], op0=ALU.mult, op1=ALU.add)
                # t2 accumulation and user-row matmul
                scrt = ep.tile([P, DIM], BF16, tag="scrt")
                nc.vector.scalar_tensor_tensor(
                    out=scrt[:], in0=hts, scalar=1.0, in1=ws2_16[:],
                    op0=ALU.mult, op1=ALU.mult, accum_out=t2col[:, j:j + 1])
                pu = psE.tile([8, DIM], F32, tag="misc")
                nc.tensor.matmul(out=pu[:], lhsT=uoh_sb[:, j * 8:(j + 1) * 8],
                                 rhs=hts, start=True, stop=True)
                nc.vector.tensor_tensor(out=pu_acc[:], in0=pu_acc[:], in1=pu[:],
                                        op=ALU.add)

            for sg0 in range(0, NT, SG):
                sgn = min(SG, NT - sg0)
                ug = ugp.tile([P, SG * DIM], F32, tag="ug")
                for ti in range(sgn):
                    nc.gpsimd.indirect_dma_start(
                        out=ug[:, ti * DIM:(ti + 1) * DIM], out_offset=None,
                        in_=t_hid[:, :],
                        in_offset=bass.IndirectOffsetOnAxis(
                            ap=ssub_sb[:, sg0 + ti:sg0 + ti + 1], axis=0))
                u16 = u16p.tile([P, SG * DIM], BF16, tag="u16")
                for ci in range(0, sgn, 4):
                    cn4 = min(4, sgn - ci)
                    nc.scalar.copy(out=u16[:, ci * DIM:(ci + cn4) * DIM],
                                   in_=ug[:, ci * DIM:(ci + cn4) * DIM])
                ohsg = ohp.tile([N_REL + 1, SG * P], BF16, tag="ohsg")
                nc.sync.dma_start(out=ohsg[:, 0:sgn * P],
                                  in_=t_oh[:, sg0 * P:(sg0 + sgn) * P])
                RW = DIM + 1
                b16f = chp.tile([P, SG * RW], BF16, tag="b16sg")
                stage = chp.tile([P, 3 * SG], F32, tag="stage")
                for ti in range(sgn):
                    # one psum tile per B-matmul (single writer, single reader)
                    pb = psB.tile([P, RW], F32, tag="pb")
                    nc.tensor.matmul(out=pb[:],
                                     lhsT=ohsg[:, ti * P:(ti + 1) * P],
                                     rhs=brhs[:], start=True, stop=True)
                    nc.scalar.copy(out=b16f[:, ti * RW:(ti + 1) * RW], in_=pb[:])
                y2v = b16f[:].rearrange("p (t c) -> p t c", c=RW)[:, 0:sgn, DIM:DIM + 1]
                nc.vector.tensor_copy(out=stage[:, 2 * SG:2 * SG + sgn], in_=y2v)
                for ti in range(sgn):
                    u1 = u16[:, ti * DIM:(ti + 1) * DIM]
                    b1 = b16f[:, ti * RW:ti * RW + DIM]
                    scr = wp.tile([P, DIM], BF16, tag="scr")
                    scr2 = wp.tile([P, DIM], BF16, tag="scr2")
                    nc.scalar.activation(out=scr2[:], in_=u1, func=AF.Square,
                                         accum_out=stage[:, ti:ti + 1])
                    nc.vector.scalar_tensor_tensor(
                        out=scr[:], in0=u1, scalar=1.0, in1=b1,
                        op0=ALU.mult, op1=ALU.mult,
                        accum_out=stage[:, SG + ti:SG + ti + 1])

                # ---- chain on [P, sgn] f32 ----
                C = chp.tile([P, 10 * SG], F32, tag="chainC")
                def cc_(i):
                    return C[:, i * SG:i * SG + sgn]
                nsq = stage[:, 0:sgn]
                xyu = stage[:, SG:SG + sgn]
                y2 = stage[:, 2 * SG:2 * SG + sgn]
                # sA poly
                nc.vector.tensor_scalar(out=cc_(0), in0=nsq, scalar1=-17.0 / 315.0,
                                        scalar2=2.0 / 15.0, op0=ALU.mult, op1=ALU.add)
                nc.vector.tensor_tensor(out=cc_(0), in0=cc_(0), in1=nsq, op=ALU.mult)
                nc.vector.tensor_scalar(out=cc_(0), in0=cc_(0), scalar1=-1.0 / 3.0,
                                        scalar2=None, op0=ALU.add)
                nc.vector.tensor_tensor(out=cc_(0), in0=cc_(0), in1=nsq, op=ALU.mult)
                nc.vector.tensor_scalar(out=cc_(0), in0=cc_(0), scalar1=1.0,
                                        scalar2=None, op0=ALU.add)        # sA
                nc.vector.tensor_tensor(out=cc_(1), in0=cc_(0), in1=xyu, op=ALU.mult)  # xy
                nc.vector.tensor_tensor(out=cc_(2), in0=cc_(0), in1=cc_(0), op=ALU.mult)
                nc.vector.tensor_tensor(out=cc_(2), in0=cc_(2), in1=nsq, op=ALU.mult)  # x2
                nc.vector.tensor_tensor(out=cc_(3), in0=cc_(2), in1=y2, op=ALU.mult)
                nc.vector.scalar_tensor_tensor(out=cc_(3), in0=cc_(1), scalar=2.0,
                                               in1=cc_(3), op0=ALU.mult, op1=ALU.add)
                nc.vector.tensor_scalar(out=cc_(3), in0=cc_(3), scalar1=1.0,
                                        scalar2=None, op0=ALU.add)        # den
                nc.vector.reciprocal(out=cc_(3), in_=cc_(3))              # rden
                nc.vector.tensor_scalar(out=cc_(4), in0=y2, scalar1=1.0,
                                        scalar2=None, op0=ALU.add)
                nc.vector.scalar_tensor_tensor(out=cc_(4), in0=cc_(1), scalar=2.0,
                                               in1=cc_(4), op0=ALU.mult, op1=ALU.add)
                nc.vector.tensor_tensor(out=cc_(4), in0=cc_(4), in1=cc_(3), op=ALU.mult)  # cA
                nc.vector.tensor_scalar(out=cc_(5), in0=cc_(2), scalar1=-1.0,
                                        scalar2=1.0, op0=ALU.mult, op1=ALU.add)
                nc.vector.tensor_tensor(out=cc_(5), in0=cc_(5), in1=cc_(3), op=ALU.mult)  # cB
                # m2 = cA^2 x2 + 2 cA cB xy + cB^2 y2
                nc.vector.tensor_tensor(out=cc_(6), in0=cc_(4), in1=cc_(4), op=ALU.mult)
                nc.vector.tensor_tensor(out=cc_(6), in0=cc_(6), in1=cc_(2), op=ALU.mult)
                nc.vector.tensor_tensor(out=cc_(7), in0=cc_(4), in1=cc_(5), op=ALU.mult)
                nc.vector.tensor_tensor(out=cc_(7), in0=cc_(7), in1=cc_(1), op=ALU.mult)
                nc.vector.scalar_tensor_tensor(out=cc_(6), in0=cc_(7), scalar=2.0,
                                               in1=cc_(6), op0=ALU.mult, op1=ALU.add)
                nc.vector.tensor_tensor(out=cc_(7), in0=cc_(5), in1=cc_(5), op=ALU.mult)
                nc.vector.tensor_tensor(out=cc_(7), in0=cc_(7), in1=y2, op=ALU.mult)
                nc.vector.tensor_tensor(out=cc_(6), in0=cc_(6), in1=cc_(7), op=ALU.add)  # m2
                nc.vector.tensor_scalar(out=cc_(6), in0=cc_(6), scalar1=0.0,
                                        scalar2=None, op0=ALU.max)
                # cc poly: 1 + w(1/3 + w(1/5 + w(1/7 + w(1/9 + w/11))))
                w_ = cc_(6)
                nc.vector.tensor_scalar(out=cc_(7), in0=w_, scalar1=1.0 / 11.0,
                                        scalar2=1.0 / 9.0, op0=ALU.mult, op1=ALU.add)
                for coef in (1.0 / 7.0, 1.0 / 5.0, 1.0 / 3.0):
                    nc.vector.tensor_tensor(out=cc_(7), in0=cc_(7), in1=w_, op=ALU.mult)
                    nc.vector.tensor_scalar(out=cc_(7), in0=cc_(7), scalar1=coef,
                                            scalar2=None, op0=ALU.add)
                nc.vector.tensor_tensor(out=cc_(7), in0=cc_(7), in1=w_, op=ALU.mult)
                nc.vector.tensor_scalar(out=cc_(7), in0=cc_(7), scalar1=1.0,
                                        scalar2=None, op0=ALU.add)        # cc
                vsl = valid_sb[:, sg0:sg0 + sgn]
                fac = chp.tile([P, 2 * SG], F32, tag="fac")
                nc.vector.tensor_tensor(out=cc_(8), in0=cc_(4), in1=cc_(7), op=ALU.mult)
                nc.vector.tensor_tensor(out=cc_(8), in0=cc_(8), in1=cc_(0), op=ALU.mult)
                nc.vector.tensor_tensor(out=fac[:, 0:sgn], in0=cc_(8), in1=vsl,
                                        op=ALU.mult)                      # fA
                nc.vector.tensor_tensor(out=cc_(9), in0=cc_(5), in1=cc_(7), op=ALU.mult)
                nc.vector.tensor_tensor(out=fac[:, SG:SG + sgn], in0=cc_(9), in1=vsl,
                                        op=ALU.mult)                      # fB

                # ---- per tile: msg, msgsq, sums; epilogue at batch end ----
                for ti in range(sgn):
                    t = sg0 + ti
                    j = int(t2b[t])
                    pos = int(t2pos[t])
                    L = int(Lb[j])
                    if pos == 0:
                        mbuf = msgp.tile([P, LMAX * 2 * DIM], BF16, tag="mbuf")
                        state["mbuf"] = mbuf
                        pwj = psW.tile([P, 2 * DIM], F32, tag="pw")
                        state.setdefault("psW", {})[j] = pwj
                    mbuf = state["mbuf"]
                    pwj = state["psW"][j]
                    u1 = u16[:, ti * DIM:(ti + 1) * DIM]
                    b1 = b16f[:, ti * RW:ti * RW + DIM]
                    t1 = wp.tile([P, DIM], BF16, tag="t1")
                    nc.vector.tensor_scalar(out=t1[:], in0=u1,
                                            scalar1=fac[:, ti:ti + 1], scalar2=None,
                                            op0=ALU.mult)
                    mslice = mbuf[:, pos * 2 * DIM:pos * 2 * DIM + DIM]
                    nc.vector.scalar_tensor_tensor(
                        out=mslice, in0=b1, scalar=fac[:, SG + ti:SG + ti + 1],
                        in1=t1[:], op0=ALU.mult, op1=ALU.add)
                    nc.scalar.activation(
                        out=mbuf[:, pos * 2 * DIM + DIM:(pos + 1) * 2 * DIM],
                        in_=mslice, func=AF.Square)
                    nc.tensor.matmul(out=pwj[:, :], lhsT=ident[:],
                                     rhs=mbuf[:, pos * 2 * DIM:(pos + 1) * 2 * DIM],
                                     start=(pos == 0), stop=(pos == L - 1))
                    if pos == L - 1:
                        batch_epilogue(j, mbuf, L)
                        del state["psW"][j]

            # ---------------- allreduce + gate + output ----------------
            nc.sync.dma_start(out=d_hu_in[:, :], in_=pu_acc[:])
            import concourse.mybir as mybir2
            nc.gpsimd.collective_compute(
                "AllReduce", mybir2.AluOpType.add,
                replica_groups=[list(range(NCORES))],
                ins=[d_hu_in[:, :]], outs=[d_hu_out[:, :]])
            hu2 = wp.tile([8, DIM], F32, tag="hu2")
            nc.sync.dma_start(out=hu2[:], in_=d_hu_out[:, :])
            hu16 = cp.tile([8, DIM], BF16)
            nc.vector.tensor_copy(out=hu16[:], in_=hu2[:])
            su = cp.tile([8, 1], F32)
            scr8 = wp.tile([8, DIM], BF16, tag="scr8")
            nc.vector.scalar_tensor_tensor(out=scr8[:], in0=hu16[:], scalar=1.0,
                                           in1=ws1_16[:], op0=ALU.mult, op1=ALU.mult,
                                           accum_out=su[:, 0:1])
            su16 = cp.tile([8, 1], BF16)
            nc.vector.tensor_copy(out=su16[:], in_=su[:])
            for j in range(NB):
                psu = psE.tile([P, 1], F32, tag="misc")
                nc.tensor.matmul(out=psu[:], lhsT=nboh_sb[:, j * P:(j + 1) * P],
                                 rhs=su16[:], start=True, stop=True)
                nc.vector.tensor_tensor(out=pre_col[:, j:j + 1], in0=t2col[:, j:j + 1],
                                        in1=psu[:], op=ALU.add)
            alpha = cp.tile([P, NB], F32)
            nc.scalar.activation(out=alpha[:], in_=pre_col[:], func=AF.Sigmoid,
                                 bias=bsc_sb[:, 0:1])
            for j in range(NB):
                ob = wp.tile([P, DIM], F32, tag="ob")
                nc.vector.tensor_scalar(out=ob[:], in0=htil[:, j * DIM:(j + 1) * DIM],
                                        scalar1=alpha[:, j:j + 1], scalar2=None,
                                        op0=ALU.mult)
                nc.sync.dma_start(out=t_out[j * P:(j + 1) * P, :], in_=ob[:])
    return nc


# ----------------------------------------------------------------------------
# entry point
# ----------------------------------------------------------------------------

def kernel(hidden, rela_embed, W_agg, b_agg, W_score, b_score,
           edges, nodes, q_sub, old_nodes_new_idx):
    import ml_dtypes
    from concourse.bass_utils import run_bass_kernel_spmd

    struct, per_core, inv, user_idx, nb_arr = preprocess(
        edges, nodes, q_sub, old_nodes_new_idx)
    NB, NT, S = struct["NB"], struct["NT"], struct["S"]

    nc = build_graph(struct)

    hidden = np.ascontiguousarray(np.asarray(hidden, np.float32))
    rela = np.ascontiguousarray(np.asarray(rela_embed, np.float32))
    W = np.asarray(W_agg, np.float32)
    # wstat[:, k*384:(k+1)*384] = [W1_k | W2_k | W3_k] for stat k
    wstat = np.zeros((P, 4 * 384), np.float32)
    for k in range(4):
        wstat[:, k * 384 + 0:k * 384 + 128] = W[k * DIM:(k + 1) * DIM]
        wstat[:, k * 384 + 128:k * 384 + 256] = W[(4 + k) * DIM:(5 + k) * DIM]
        wstat[:, k * 384 + 256:k * 384 + 384] = W[(8 + k) * DIM:(9 + k) * DIM]
    w13 = np.ascontiguousarray(W[12 * DIM:13 * DIM])
    bagg = np.asarray(b_agg, np.float32).reshape(1, DIM)
    ws = np.asarray(W_score, np.float32)
    ws1rep = np.repeat(ws[0:DIM, 0][None, :], 8, axis=0)
    ws2rep = np.repeat(ws[DIM:2 * DIM, 0][None, :], P, axis=0)
    bscore_col = np.full((P, 1), np.asarray(b_score, np.float32)[0], np.float32)

    in_maps = []
    for c in range(NCORES):
        pc = per_core[c]
        ohrel = np.zeros((N_REL + 1, S), dtype=ml_dtypes.bfloat16)
        sr = pc["srel"].T.reshape(-1)          # slot order: tile-major, partition fast
        # slot s of tile t lives at column t*P+p ; srel is [P, NT]
        cols = np.arange(S)
        ohrel[sr.reshape(NT, P).reshape(-1), cols] = 1.0
        nboh = np.zeros((8, NB * P), dtype=ml_dtypes.bfloat16)
        uoh = np.zeros((P, NB * 8), dtype=ml_dtypes.bfloat16)
        nid = pc["node_id"]
        ok = nid >= 0
        nboh[nb_arr[nid[ok]], np.where(ok)[0]] = 1.0
        for b in range(BATCH):
            wpos = np.where(nid == user_idx[b])[0]
            if len(wpos):
                n = int(wpos[0])
                uoh[n % P, (n // P) * 8 + b] = 1.0
        in_maps.append({
            "hidden": hidden, "slot_sub": pc["ssub"], "valid": pc["valid"],
            "ohrel": np.asarray(ohrel), "rela": rela, "wstat": wstat,
            "w13": w13, "bagg": bagg, "ws1rep": ws1rep, "ws2rep": ws2rep,
            "bscore": bscore_col, "degb": pc["deg"], "corrb": pc["corr"],
            "hprev_idx": pc["hpidx"], "hprev_msk": pc["hpmask"],
            "nboh": np.asarray(nboh), "uoh": np.asarray(uoh),
        })

    do_trace = bool(int(os.environ.get("KERNEL_TRACE", "0")))
    if do_trace:
        _ensure_ntff_hook()
    res = run_bass_kernel_spmd(nc, in_maps, core_ids=list(range(NCORES)),
                               trace=do_trace,
                               tmpdir=os.environ.get("KERNEL_TRACE_DIR"))
    kernel.last_exec_time_ns = res.exec_time_ns

    out = np.zeros((N_NODES, DIM), dtype=np.float32)
    for c in range(NCORES):
        oc = res.results[c]["out"]
        nid = per_core[c]["node_id"]
        ok = nid >= 0
        out[nid[ok]] = oc[ok]
    return out
